# revision 1
# baseline (speedup 1.0000x reference)
"""GNN message-passing on 8 TRN2 NeuronCores — bf16 pair-table streamed gather.

The gather table packs TWO nodes per 256B row: xpr[q] = [bf16(x[2q]) |
bf16(x[2q+1])] ([25000, 128] bf16). Each edge gathers row src>>1 — halving
the row space doubles HBM row-buffer locality for the per-bucket sorted
index streams (gap ~12 rows = 3KB), and pair ids fit int16 with a single
table (no source-half split, so gather calls stream across the whole tile
sequence). Buckets are (dst block, src&1); tiles are parity-pure and each
needs one matmul with lhsT = the 64-column half selected by the tile's
parity. Values are plain bf16 (no hi/lo split): segment sums accumulate
exactly in f32 PSUM, so the only error is the input rounding (~1e-3 L2,
gate is 2e-2).
"""

import numpy as np
import ml_dtypes

import concourse.tile as tile
from concourse import bacc, mybir
from concourse import bass_utils

N_NODES = 50000
D = 64
N_CORES = 8
NODES_PER_CORE = N_NODES // N_CORES  # 6250
BLOCK = 128
N_PAIRS = N_NODES // 2
PAD_REL = 999.0
CHUNK_T = 16


def assign_nodes(deg_ev, deg_od, n_cores, n_blocks, block):
    n_nodes = len(deg_ev)
    nb = n_cores * n_blocks
    ev_s = np.zeros(nb)
    od_s = np.zeros(nb)
    cnt = np.zeros(nb, np.int64)
    assign = np.empty(n_nodes, np.int64)
    order = np.argsort(-(deg_ev + deg_od), kind="stable")
    full_penalty = np.zeros(nb)
    for n in order:
        score = np.maximum(ev_s + deg_ev[n], od_s + deg_od[n]) + full_penalty
        b = int(np.argmin(score))
        assign[n] = b
        ev_s[b] += deg_ev[n]
        od_s[b] += deg_od[n]
        cnt[b] += 1
        if cnt[b] >= block:
            full_penalty[b] = 1e18
    core_of = assign // n_blocks
    block_of = assign % n_blocks
    pos_of = np.empty(n_nodes, np.int64)
    fill = np.zeros(nb, np.int64)
    for n in order:
        b = assign[n]
        pos_of[n] = fill[b]
        fill[b] += 1
    return core_of, block_of, pos_of


def bin_edges(edge_index, n_cores=N_CORES, nodes_per_core=NODES_PER_CORE,
              block=BLOCK, n_blocks=None):
    dst = np.asarray(edge_index[0], dtype=np.int64)
    src = np.asarray(edge_index[1], dtype=np.int64)
    n_nodes = n_cores * nodes_per_core
    if n_blocks is None:
        n_blocks = -(-nodes_per_core // block) + 1

    par = src & 1
    pair = src >> 1
    deg_ev = np.bincount(dst[par == 0], minlength=n_nodes)
    deg_od = np.bincount(dst[par == 1], minlength=n_nodes)
    core_of, block_of, pos_of = assign_nodes(
        deg_ev, deg_od, n_cores, n_blocks, block
    )

    core = core_of[dst]
    blk = block_of[dst]
    rel = pos_of[dst].astype(np.float32)
    bucket = (core * n_blocks + blk) * 2 + par
    order = np.lexsort((pair, bucket))
    pair_s = pair[order]
    rel_s = rel[order]

    counts = np.bincount(bucket, minlength=n_cores * n_blocks * 2).reshape(
        n_cores, n_blocks, 2
    )
    T = -(-counts.max(axis=0) // 128)  # [n_blocks, 2]
    T[:, 0] = np.maximum(1, T[:, 0])
    tot_tiles = int(T.sum())
    tot_edges = tot_tiles * 128

    pad_sizes = T.reshape(-1) * 128
    pad_starts = np.zeros(n_blocks * 2, dtype=np.int64)
    pad_starts[1:] = np.cumsum(pad_sizes)[:-1]

    src_pad = np.zeros((n_cores, tot_edges), dtype=np.int16)
    rel_pad = np.full((n_cores, tot_edges), PAD_REL, dtype=np.float32)

    cum = counts.reshape(n_cores, -1).cumsum(axis=1)
    starts_real = np.zeros((n_cores, n_blocks * 2), dtype=np.int64)
    starts_real[:, 1:] = cum[:, :-1]
    core_base = np.zeros(n_cores, dtype=np.int64)
    core_counts = counts.sum(axis=(1, 2))
    core_base[1:] = np.cumsum(core_counts)[:-1]

    for c in range(n_cores):
        cnts = counts[c].reshape(-1)
        for bh in range(n_blocks * 2):
            n = int(cnts[bh])
            if n == 0:
                continue
            s = int(core_base[c] + starts_real[c, bh])
            p = int(pad_starts[bh])
            w = int(pad_sizes[bh])
            src_pad[c, p : p + n] = pair_s[s : s + n].astype(np.int16)
            # pad slots re-read the last real row (same-row HBM hit)
            src_pad[c, p + n : p + w] = pair_s[s + n - 1]
            rel_pad[c, p : p + n] = rel_s[s : s + n]

    w = src_pad.reshape(n_cores, -1, 16).transpose(0, 2, 1)
    src16 = np.tile(w, (1, 8, 1)).copy()

    dstrel = (
        rel_pad.reshape(n_cores, -1, 128)
        .transpose(0, 2, 1)
        .astype(ml_dtypes.bfloat16)
        .copy()
    )

    return T, src16, dstrel, (core_of, block_of, pos_of)


def make_pair_table(x):
    """x [N, 64] f32 -> [N/2, 128] bf16: row q = [bf16(x[2q]) | bf16(x[2q+1])]."""
    return x.astype(ml_dtypes.bfloat16).reshape(N_NODES // 2, 2 * D).copy()


def build_program(T, d=D, block=BLOCK, repeat=1, msgs_bufs=8, sel_bufs=8,
                  psum_bufs=8, chunk_t=CHUNK_T, n_queues=4, sel_pool_frac=0,
                  gather_only=False, no_sel=False):
    n_blocks = T.shape[0]
    out_cols = n_blocks * block
    tot_tiles = int(T.sum())
    d2 = 2 * d  # bf16 cols per pair row

    nc = bacc.Bacc(
        "TRN2",
        target_bir_lowering=False,
        debug=False,
        num_devices=N_CORES,
        num_swdge_queues=4,
    )
    xpr = nc.dram_tensor(
        "xpr", [N_PAIRS, d2], mybir.dt.bfloat16, kind="ExternalInput"
    )
    src16 = nc.dram_tensor(
        "src16", [128, tot_tiles * 8], mybir.dt.int16, kind="ExternalInput"
    )
    dstrel = nc.dram_tensor(
        "dstrel", [128, tot_tiles], mybir.dt.bfloat16, kind="ExternalInput"
    )
    iota_in = nc.dram_tensor(
        "iota", [128, block], mybir.dt.bfloat16, kind="ExternalInput"
    )
    out = nc.dram_tensor("out", [d, out_cols], mybir.dt.float32, kind="ExternalOutput")

    par_of_tile = []
    blk_of_tile = []
    blk_start = []
    blk_end = []
    abs_t = 0
    for b in range(n_blocks):
        blk_start.append(abs_t)
        for parity in range(2):
            for _ in range(int(T[b, parity])):
                par_of_tile.append(parity)
                blk_of_tile.append(b)
                abs_t += 1
        blk_end.append(abs_t)
    assert abs_t == tot_tiles

    with tile.TileContext(nc) as tc:
        with (
            tc.tile_pool(name="meta", bufs=1) as meta_pool,
            tc.tile_pool(name="msgs", bufs=msgs_bufs) as msgs_pool,
            tc.tile_pool(name="sel", bufs=sel_bufs) as sel_pool,
            tc.tile_pool(name="obuf", bufs=2) as obuf_pool,
            tc.tile_pool(name="psum", bufs=psum_bufs, space="PSUM") as psum_pool,
        ):
            src_t = meta_pool.tile([128, tot_tiles * 8], mybir.dt.int16)
            nc.sync.dma_start(src_t[:], src16.ap())
            rel_t = meta_pool.tile([128, tot_tiles], mybir.dt.bfloat16)
            nc.sync.dma_start(rel_t[:], dstrel.ap())
            iota_t = meta_pool.tile([128, block], mybir.dt.bfloat16)
            nc.sync.dma_start(iota_t[:], iota_in.ap())

            def body():
                outbuf = obuf_pool.tile([d, out_cols], mybir.dt.float32, tag="ob")
                if gather_only:
                    nc.vector.memset(outbuf[:], 0.0)
                q = 0
                ci = 0
                psum = None
                for k0 in range(0, tot_tiles, chunk_t):
                    tn = min(chunk_t, tot_tiles - k0)
                    msgs = msgs_pool.tile(
                        [128, chunk_t, d2], mybir.dt.bfloat16, tag="msgs"
                    )
                    nc.gpsimd.dma_gather(
                        msgs[:, 0:tn, :],
                        xpr.ap(),
                        src_t[:, k0 * 8 : (k0 + tn) * 8],
                        tn * 128,
                        tn * 128,
                        d2,
                        queue_num=q % n_queues,
                        single_packet=False,
                    )
                    q += 1
                    if gather_only:
                        continue

                    if not no_sel:
                        sel = sel_pool.tile(
                            [128, chunk_t, block], mybir.dt.bfloat16, tag="sel"
                        )
                        # optionally offload a fraction of sel builds to gpsimd
                        eng = (
                            nc.gpsimd
                            if sel_pool_frac and (ci % sel_pool_frac == 0)
                            else nc.vector
                        )
                        ci += 1
                        eng.tensor_tensor(
                            out=sel[:, 0:tn, :],
                            in0=rel_t[:, k0 : k0 + tn].to_broadcast(
                                [128, tn, block]
                            ),
                            in1=iota_t[:]
                            .rearrange("p (o n) -> p o n", o=1)
                            .to_broadcast([128, tn, block]),
                            op=mybir.AluOpType.is_equal,
                        )

                    for t in range(k0, k0 + tn):
                        b = blk_of_tile[t]
                        parity = par_of_tile[t]
                        if t == blk_start[b]:
                            psum = psum_pool.tile(
                                [d, block], mybir.dt.float32, space="PSUM"
                            )
                        nc.tensor.matmul(
                            out=psum[:],
                            lhsT=msgs[:, t - k0, parity * d : (parity + 1) * d],
                            rhs=iota_t[:] if no_sel else sel[:, t - k0, :],
                            start=(t == blk_start[b]),
                            stop=(t == blk_end[b] - 1),
                        )
                        if t == blk_end[b] - 1:
                            nc.scalar.mul(
                                outbuf[:, b * block : (b + 1) * block],
                                psum[:],
                                1.0,
                            )
                nc.sync.dma_start(out.ap(), outbuf[:])

            if repeat > 1:
                with tc.For_i(0, repeat, 1):
                    body()
            else:
                body()

    nc.compile()
    return nc


def make_iota():
    return np.broadcast_to(
        np.arange(BLOCK, dtype=np.float32)[None, :], (128, BLOCK)
    ).astype(ml_dtypes.bfloat16).copy()


def unshard_output(results, node_loc, block=BLOCK, n_nodes=N_NODES, d=D):
    core_of, block_of, pos_of = node_loc
    cols = block_of * block + pos_of
    out = np.empty((n_nodes, d), dtype=np.float32)
    for c in range(len(results)):
        mask = core_of == c
        out[mask] = results[c]["out"].T[cols[mask]]
    return out


def prep_inputs(inputs):
    edge_index = np.asarray(inputs["edge_index"])
    x = np.ascontiguousarray(np.asarray(inputs["x"], np.float32))
    T, src16, dstrel, node_loc = bin_edges(edge_index)
    xpr = make_pair_table(x)
    iota = make_iota()
    in_maps = [
        {"xpr": xpr, "src16": src16[c], "dstrel": dstrel[c], "iota": iota}
        for c in range(N_CORES)
    ]
    return (T,), in_maps, node_loc


def build(build_args, repeat=1):
    (T,) = build_args
    return build_program(T, repeat=repeat)


def kernel(edge_index, x):
    build_args, in_maps, node_loc = prep_inputs(
        {"edge_index": edge_index, "x": x}
    )
    nc = build(build_args)
    res = bass_utils.run_bass_kernel_spmd(nc, in_maps, core_ids=list(range(N_CORES)))
    return unshard_output(res.results, node_loc)

